# revision 23
# baseline (speedup 1.0000x reference)
"""AnchorTransformer kernel for 8 TRN2 NeuronCores.

Data-parallel over the flattened pixel dim N = B*H*W = 32768 -> 4096/core,
with pixels HOST-SORTED by instance id (stable argsort of max(lab-1,0)).
After sorting, any 512-pixel sub-block spans <= 3 distinct instances
(measured max 3 on the fixed-seed inputs; capacity 4), so per sub-block
only 32 anchor rows (4 instances x 8 anchors) are needed instead of the
full 512 -- the dense-scores-over-all-anchors baseline wastes 16x PE work
and 3x HBM traffic on it.

Math per pixel n (instance i = max(lab-1,0)):
    q = f W_q^T + b_q;  s_j = scale * q.K_j  over the 32 in-block slots
    masked softmax over i's 8 slots; o = attn @ V2 (out-proj folded);
    host: divide by denom, gate background, residual add, unsort.

Host-folded replicated tables (per the sharding hint):
    KW   = scale * (A W_k^T + b_k) W_q          (J, C)  score weights
    sb_j = scale * (A W_k^T + b_k)_j . b_q      (J,)    exp bias
    V2   = (A W_v^T + b_v) W_o^T + 1 (x) b_o    (J, C)  out-proj folded,
           plus a ones column emitting the softmax denominator.
    mask = +30 on the pixel's own instance slots (softmax shift invariance;
           e^-30 leakage ~ 1e-13), via ONE K=16 block-diagonal matmul.

Partition packing (engine cost ~ free-dim columns; partitions are free):
4 sub-blocks of 512 px live at partition offsets 32g of ONE PSUM tile
[128, 512].  Per 2048-px group: 8 score matmuls (K=128, M=32, col-group
tiled -> 4-way concurrent), 1 mask matmul (K=16, M=128), ONE exp
activation [128, 512] with per-partition bias, 16 attention matmuls
(K=32, row-group tiled, pairs writing adjacent PSUM banks), 8 paired
strided-AP PSUM->SBUF bf16 copies split scalar/vector, one bf16 DMA per
group of unnormalized o + denom.  DMA instruction count is minimized
(11/core) so the tile framework never reuses a DMA semaphore -- reuse
preconditions serialized transfers in earlier revisions.
"""

from contextlib import ExitStack

import numpy as np
import ml_dtypes
import concourse.bass as bass
import concourse.tile as tile
from concourse import bacc, mybir
from concourse.bass_utils import run_bass_kernel_spmd

NCORES = 8
N_FULL = 32768
NP = N_FULL // NCORES  # 4096 pixels per core
C = 256
M_INST = 64
L = 8
J = M_INST * L          # 512 anchor rows total
TP = 512                # pixels per sub-block
G = 4                   # sub-blocks packed per group (partition dim)
NMT = NP // (TP * G)    # 2 groups per core
NI = 4                  # instance capacity per sub-block
NSLOT = NI * L          # 32 anchor slots per sub-block
F32 = mybir.dt.float32
BF16 = mybir.dt.bfloat16
SCALE = 1.0 / 16.0
BIG = 30.0

AF = mybir.ActivationFunctionType


def build_nc():
    nc = bacc.Bacc()
    fT = nc.declare_dram_parameter("fT", [C, NP], BF16, isOutput=False)
    KT = nc.declare_dram_parameter("KT", [128, NMT * G * 2 * NSLOT], BF16,
                                   isOutput=False)
    # V2 rows ++ bit-packed f32 exp-bias (2 f32 = 4 bf16 cols); one DMA --
    # a separate [128, 2] f32 param would be 128 8-byte DMA packets that
    # poison the queue for ~7us
    V2R = nc.declare_dram_parameter("V2R", [128, NMT * 257 + 2 * NMT], BF16,
                                    isOutput=False)
    # EB one-hot [16, NMT*TP] ++ block-diag mask lhsT RC [16, 128]
    EBRC = nc.declare_dram_parameter("EBRC", [G * NI, NMT * TP + 128], BF16,
                                     isOutput=False)
    OB = nc.declare_dram_parameter("OB", [NMT, 128, 16 * 257], BF16,
                                   isOutput=True)

    with tile.TileContext(nc) as tc, ExitStack() as es:
        cp = es.enter_context(tc.tile_pool(name="const", bufs=1))
        io = es.enter_context(tc.tile_pool(name="io", bufs=4))
        # single score bank: forces the static scheduler to order group
        # mt+1's score matmuls after group mt's exp has drained the bank
        sps = es.enter_context(tc.tile_pool(name="sps", space="PSUM", bufs=1))
        ops = es.enter_context(tc.tile_pool(name="ops", space="PSUM", bufs=3))

        # PE warmup from a memset tile: starts inside the framework
        # preamble shadow, no DMA dependency; ramps HAM before the real
        # matmuls. Sink DMA (end of gpsimd queue) defeats DCE.
        wsrc = cp.tile([128, TP], BF16, tag="wsrc")
        nc.gpsimd.memset(wsrc[:], 0.5)
        wps = sps.tile([128, TP], F32, tag="s", bufs=1)
        for w in range(5):
            nc.tensor.matmul(wps[:], wsrc[:, 0:128], wsrc[:],
                             start=True, stop=True, skip_group_check=True)
        wsink = io.tile([1, TP], F32, tag="wsink")
        nc.vector.tensor_copy(wsink[:], wps[0:1, :])
        warm_dram = nc.dram_tensor("warm_sink", [1, TP], F32)

        # tables on the scalar queue, compute-order
        V2_sb = cp.tile([128, NMT * 257 + 2 * NMT], BF16, tag="v2")
        nc.scalar.dma_start(V2_sb[:], V2R[:, :])
        EB_sb = cp.tile([G * NI, NMT * TP + 128], BF16, tag="eb")
        nc.scalar.dma_start(EB_sb[:], EBRC[:, :])

        def sbj_ap(mt):  # [128, 1] f32 view of the bit-packed bias
            c0 = NMT * 257 + 2 * mt
            return V2_sb[:, c0:c0 + 2].bitcast(F32)

        # score weights + features: sync h=0, gpsimd h=1
        KT_sb = cp.tile([128, NMT * G * 2 * NSLOT], BF16, tag="kt")
        nc.sync.dma_start(KT_sb[:], KT[:, :])
        # fT tiles recycle one buffer per (h, gp) across groups: group
        # mt+1's load physically cannot post until group mt's scores have
        # consumed the buffer -- staggers the in-phase so mt0's chunks get
        # the full DMA bandwidth (the engines round-robin over ALL posted
        # descriptors, so posting everything up front starves the head)
        fT_t = {}
        for mt in range(NMT):
            for h in range(2):
                for gp in range(2):
                    t = io.tile([128, 2 * TP], BF16, tag=f"ft{h}{gp}",
                                bufs=1)
                    fT_t[(mt, h, gp)] = t
                    c0 = (mt * 2 + gp) * 2 * TP
                    (nc.sync if h == 0 else nc.gpsimd).dma_start(
                        t[:], fT[h * 128:(h + 1) * 128, c0:c0 + 2 * TP])

        for mt in range(NMT):
            sp = sps.tile([128, TP], F32, tag="s", bufs=1)
            # scores: col-group tiled (one col group per sub-block g);
            # chunk-pair order tracks the fT DMA arrival order
            for gp in range(2):
                for h in range(2):
                    for g in (2 * gp, 2 * gp + 1):
                        col = ((mt * G + g) * 2 + h) * NSLOT
                        nc.tensor.matmul(
                            sp[32 * g:32 * g + 32, :],
                            KT_sb[:, col:col + NSLOT],
                            fT_t[(mt, h, gp)][:, (g % 2) * TP:
                                              (g % 2 + 1) * TP],
                            start=(h == 0), stop=False,
                            tile_position=(0, 32 * g),
                            skip_group_check=True)
            # +BIG mask on own-instance slots: one K=16 block-diag matmul
            nc.tensor.matmul(
                sp[:], EB_sb[:, NMT * TP:], EB_sb[:, mt * TP:(mt + 1) * TP],
                start=False, stop=True, tile_position=(0, 0),
                skip_group_check=True)
            # one exp for all 2048 pixels of the group
            P = io.tile([128, TP], BF16, tag="p", bufs=2)
            nc.scalar.activation(P[:], sp[:], AF.Exp, bias=sbj_ap(mt))
            # attention: per pixel-chunk c, two pairs of row-group
            # concurrent matmuls into adjacent PSUM banks; paired
            # strided-AP copies (split vector/scalar) convert to bf16
            otb = io.tile([128, 16 * 257], BF16, tag="otb", bufs=2)
            for c in range(4):
                for gp in range(2):
                    op2 = ops.tile([128, 1024], F32, tag="o", bufs=3)
                    for k in range(2):
                        g = 2 * gp + k
                        nc.tensor.matmul(
                            op2[:, 512 * k:512 * k + 257],
                            P[32 * g:32 * g + 32, c * 128:(c + 1) * 128],
                            V2_sb[32 * g:32 * g + 32,
                                  mt * 257:(mt + 1) * 257],
                            start=True, stop=True,
                            tile_position=(32 * g, 0))
                    src = op2[:].rearrange("p (b x) -> p b x", b=2)[:, :, 0:257]
                    base = (c * 4 + 2 * gp) * 257
                    dst = otb[:, base:base + 514].rearrange(
                        "p (b x) -> p b x", b=2)
                    if (c * 2 + gp) % 2 == 0:
                        nc.vector.tensor_copy(dst, src)
                    else:
                        nc.scalar.copy(dst, src)
                # drain each ready quarter (mt0: halves) off-chip early
                if mt == 0 and c in (1, 3):
                    b0, b1 = (c - 1) * 4 * 257, (c + 1) * 4 * 257
                    nc.gpsimd.dma_start(OB[mt, :, b0:b1], otb[:, b0:b1])
                elif mt > 0:
                    b0, b1 = c * 4 * 257, (c + 1) * 4 * 257
                    nc.gpsimd.dma_start(OB[mt, :, b0:b1], otb[:, b0:b1])
        nc.scalar.dma_start(warm_dram[:, :], wsink[:])

    nc.compile()
    return nc


_CACHE = {}


def _build():
    if "nc" not in _CACHE:
        _CACHE["nc"] = build_nc()
    return _CACHE["nc"]


def _prep_maps(anchors, features, instances_in_view, in_proj_w, in_proj_b,
               out_w, out_b):
    f32 = np.float32
    bf16 = ml_dtypes.bfloat16
    anchors = np.asarray(anchors, f32)
    features = np.asarray(features, f32)
    iiv = np.asarray(instances_in_view, np.int32)
    in_proj_w = np.asarray(in_proj_w, f32)
    in_proj_b = np.asarray(in_proj_b, f32)
    out_w = np.asarray(out_w, f32)
    out_b = np.asarray(out_b, f32)

    f_flat = features.reshape(N_FULL, C)
    lab = iiv.reshape(-1)
    idx = np.maximum(lab - 1, 0)
    perm = np.argsort(idx, kind="stable")
    idx_s = idx[perm]
    f_s = f_flat[perm]
    fT_full = np.ascontiguousarray(f_s.T.astype(bf16))  # (C, N) sorted

    # replicated folded tables (see module docstring)
    A = anchors.reshape(J, C)
    Wq, Wk, Wv = in_proj_w[:C], in_proj_w[C:2 * C], in_proj_w[2 * C:]
    bq, bk, bv = in_proj_b[:C], in_proj_b[C:2 * C], in_proj_b[2 * C:]
    K_all = A @ Wk.T + bk                       # (J, C)
    KW = f32(SCALE) * (K_all @ Wq)              # (J, C)
    sb = f32(SCALE) * (K_all @ bq)              # (J,)
    V2_h = (A @ Wv.T + bv) @ out_w.T + out_b    # (J, C)

    NSB = N_FULL // TP  # 64 sub-blocks globally
    rows = np.zeros((NSB, NSLOT), np.int64)     # anchor row per slot
    valid = np.zeros((NSB, NSLOT), bool)
    uniq_pad = np.full((NSB, NI), -1, np.int64)
    for b in range(NSB):
        w = idx_s[b * TP:(b + 1) * TP]
        u = np.unique(w)
        assert len(u) <= NI, f"sub-block {b} spans {len(u)} instances"
        uniq_pad[b, :len(u)] = u
        for i, inst in enumerate(u):
            rows[b, i * L:(i + 1) * L] = inst * L + np.arange(L)
            valid[b, i * L:(i + 1) * L] = True

    KW_slot = KW[rows] * valid[:, :, None]          # (NSB, 32, C)
    sb_slot = sb[rows] * valid                      # (NSB, 32)
    V2_slot = np.zeros((NSB, NSLOT, 257), f32)
    V2_slot[:, :, :256] = V2_h[rows] * valid[:, :, None]
    V2_slot[:, :, 256] = valid

    # one-hot instance membership per pixel (4 rows per sub-block)
    eb = (idx_s.reshape(NSB, 1, TP) ==
          uniq_pad[:, :, None]).astype(bf16)        # (NSB, NI, TP)

    # block-diagonal mask lhsT: RC[4g+i, 32g'+s] = BIG iff g==g', s//8==i
    RC_h = np.zeros((G * NI, 128), f32)
    for g in range(G):
        for i in range(NI):
            RC_h[NI * g + i, 32 * g + i * L:32 * g + (i + 1) * L] = BIG
    RC_h = RC_h.astype(bf16)

    in_maps = []
    for ci in range(NCORES):
        sl = slice(ci * NP, (ci + 1) * NP)
        bsl = slice(ci * (NP // TP), (ci + 1) * (NP // TP))  # 8 sub-blocks
        kw_c = KW_slot[bsl].reshape(NMT, G, NSLOT, 2, 128)   # (mt,g,s,h,p)
        KT_h = np.ascontiguousarray(
            kw_c.transpose(4, 0, 1, 3, 2).reshape(128, NMT * G * 2 * NSLOT)
        ).astype(bf16)
        v2_c = V2_slot[bsl].reshape(NMT, G, NSLOT, 257)
        V2R_h = np.ascontiguousarray(
            v2_c.transpose(1, 2, 0, 3).reshape(128, NMT * 257)).astype(bf16)
        sb_c = sb_slot[bsl].reshape(NMT, G, NSLOT)
        SBJ_h = np.ascontiguousarray(
            sb_c.transpose(1, 2, 0).reshape(128, NMT).astype(f32))
        # bit-pack the f32 bias as trailing bf16 columns of V2R
        SBJ_bits = SBJ_h.view(np.uint16).view(bf16)          # [128, 2*NMT]
        eb_c = eb[bsl].reshape(NMT, G, NI, TP)
        EB_h = np.ascontiguousarray(
            eb_c.transpose(1, 2, 0, 3).reshape(G * NI, NMT * TP))
        in_maps.append({
            "fT": np.ascontiguousarray(fT_full[:, sl]),
            "KT": KT_h,
            "V2R": np.concatenate([V2R_h, SBJ_bits], axis=1),
            "EBRC": np.concatenate([EB_h, RC_h], axis=1),
        })
    ctx = {"perm": perm, "f_s": f_s, "lab_s": lab[perm],
           "shape": features.shape}
    return in_maps, ctx


def _run(in_maps, **kw):
    nc = _build()
    return run_bass_kernel_spmd(nc, in_maps, core_ids=list(range(NCORES)),
                                **kw)


def kernel(**inputs):
    in_maps, ctx = _prep_maps(**inputs)
    res = _run(in_maps)
    o_parts = []
    for r in res.results:
        ob = np.asarray(r["OB"]).reshape(NMT, 128, 4, G, 257)
        o_parts.append(np.ascontiguousarray(
            ob.transpose(0, 3, 2, 1, 4)).reshape(NP, 257))
    o_cat = np.concatenate(o_parts, axis=0).astype(np.float32)
    o = o_cat[:, :256] / o_cat[:, 256:257]
    res_s = ctx["f_s"] + (ctx["lab_s"] > 0)[:, None] * o
    out = np.empty_like(res_s)
    out[ctx["perm"]] = res_s
    return out.reshape(ctx["shape"]).astype(np.float32)


# revision 29
# speedup vs baseline: 1.0226x; 1.0226x over previous
"""AnchorTransformer kernel for 8 TRN2 NeuronCores.

Data-parallel over the flattened pixel dim N = B*H*W = 32768 -> 4096/core,
with pixels HOST-SORTED by instance id (stable argsort of max(lab-1,0)).
After sorting, any 512-pixel sub-block spans <= 3 distinct instances
(measured max 3 on the fixed-seed inputs; capacity 4), so per sub-block
only 32 anchor rows (4 instances x 8 anchors) are needed instead of the
full 512 -- the dense-scores-over-all-anchors baseline wastes 16x PE work
and 3x HBM traffic on it.

Math per pixel n (instance i = max(lab-1,0)):
    q = f W_q^T + b_q;  s_j = scale * q.K_j  over the 32 in-block slots
    masked softmax over i's 8 slots; o = attn @ V2 (out-proj folded);
    host: divide by denom, gate background, residual add, unsort.

Host-folded replicated tables (per the sharding hint):
    KW   = scale * (A W_k^T + b_k) W_q          (J, C)  score weights
    sb_j = scale * (A W_k^T + b_k)_j . b_q      (J,)    exp bias
    V2   = (A W_v^T + b_v) W_o^T + 1 (x) b_o    (J, C)  out-proj folded,
           plus a ones column emitting the softmax denominator.
    mask = +30 on the pixel's own instance slots (softmax shift invariance;
           e^-30 leakage ~ 1e-13), via ONE K=16 block-diagonal matmul.

Partition packing (engine cost ~ free-dim columns; partitions are free):
4 sub-blocks of 512 px live at partition offsets 32g of ONE PSUM tile
[128, 512].  Per 2048-px group: 8 score matmuls (K=128, M=32, col-group
tiled -> 4-way concurrent), 1 mask matmul (K=16, M=128), ONE exp
activation [128, 512] with per-partition bias, 16 attention matmuls
(K=32, row-group tiled, pairs writing adjacent PSUM banks), 8 paired
strided-AP PSUM->SBUF bf16 copies split scalar/vector, one bf16 DMA per
group of unnormalized o + denom.  DMA instruction count is minimized
(11/core) so the tile framework never reuses a DMA semaphore -- reuse
preconditions serialized transfers in earlier revisions.
"""

from contextlib import ExitStack

import numpy as np
import ml_dtypes
import concourse.bass as bass
import concourse.tile as tile
from concourse import bacc, mybir
from concourse.bass_utils import run_bass_kernel_spmd

NCORES = 8
N_FULL = 32768
NP = N_FULL // NCORES  # 4096 pixels per core
C = 256
M_INST = 64
L = 8
J = M_INST * L          # 512 anchor rows total
TP = 512                # pixels per sub-block
G = 4                   # sub-blocks packed per group (partition dim)
NMT = NP // (TP * G)    # 2 groups per core
NI = 4                  # instance capacity per sub-block
NSLOT = NI * L          # 32 anchor slots per sub-block
F32 = mybir.dt.float32
BF16 = mybir.dt.bfloat16
SCALE = 1.0 / 16.0
BIG = 30.0

AF = mybir.ActivationFunctionType


def build_nc():
    nc = bacc.Bacc()
    fT = nc.declare_dram_parameter("fT", [C, NP], BF16, isOutput=False)
    # KT score weights [128, 512] ++ V2 rows [128, 514] ++ bit-packed f32
    # exp-bias (2 f32 = 4 bf16 cols).  One DMA with 2KB lines: the DMA
    # sequencer is per-packet-bound, so big per-partition lines are the
    # only way to reach full bandwidth (8-byte lines poison a queue ~7us).
    KTV2 = nc.declare_dram_parameter(
        "KTV2", [128, NMT * G * 2 * NSLOT + NMT * 257 + 2 * NMT], BF16,
        isOutput=False)
    # EB one-hot [16, NMT*TP] ++ block-diag mask lhsT RC [16, 128]
    EBRC = nc.declare_dram_parameter("EBRC", [G * NI, NMT * TP + 128], BF16,
                                     isOutput=False)
    OB = nc.declare_dram_parameter("OB", [NMT, 128, 16 * 257], BF16,
                                   isOutput=True)

    with tile.TileContext(nc) as tc, ExitStack() as es:
        cp = es.enter_context(tc.tile_pool(name="const", bufs=1))
        io = es.enter_context(tc.tile_pool(name="io", bufs=4))
        # single score bank: forces the static scheduler to order group
        # mt+1's score matmuls after group mt's exp has drained the bank
        sps = es.enter_context(tc.tile_pool(name="sps", space="PSUM", bufs=1))
        ops = es.enter_context(tc.tile_pool(name="ops", space="PSUM", bufs=3))

        # PE warmup from a memset tile: starts inside the framework
        # preamble shadow, no DMA dependency; ramps HAM before the real
        # matmuls. Sink DMA (end of gpsimd queue) defeats DCE.
        wsrc = cp.tile([128, TP], BF16, tag="wsrc")
        nc.gpsimd.memset(wsrc[:], 0.5)
        wps = sps.tile([128, TP], F32, tag="s", bufs=1)
        for w in range(5):
            nc.tensor.matmul(wps[:], wsrc[:, 0:128], wsrc[:],
                             start=True, stop=True, skip_group_check=True)
        wsink = io.tile([1, TP], F32, tag="wsink")
        nc.vector.tensor_copy(wsink[:], wps[0:1, :])
        warm_dram = nc.dram_tensor("warm_sink", [1, TP], F32)

        # tables on the scalar queue, compute-order
        KV_sb = cp.tile([128, NMT * G * 2 * NSLOT + NMT * 257 + 2 * NMT],
                        BF16, tag="ktv2")
        nc.scalar.dma_start(KV_sb[:], KTV2[:, :])
        EB_sb = cp.tile([G * NI, NMT * TP + 128], BF16, tag="eb")
        nc.scalar.dma_start(EB_sb[:], EBRC[:, :])
        KT_sb = KV_sb[:, :NMT * G * 2 * NSLOT]
        V2_0 = NMT * G * 2 * NSLOT

        def sbj_ap(mt):  # [128, 1] f32 view of the bit-packed bias
            c0 = V2_0 + NMT * 257 + 2 * mt
            return KV_sb[:, c0:c0 + 2].bitcast(F32)

        # features: two whole-half loads with 8KB lines (sequencer is
        # packet-bound: fat lines = full bandwidth)
        fT_t = {}
        for h in range(2):
            t = io.tile([128, NP], BF16, tag=f"ft{h}", bufs=1)
            (nc.sync if h == 0 else nc.gpsimd).dma_start(
                t[:], fT[h * 128:(h + 1) * 128, :])
            fT_t[h] = t

        for mt in range(NMT):
            sp = sps.tile([128, TP], F32, tag="s", bufs=1)
            # scores: col-group tiled (one col group per sub-block g)
            for h in range(2):
                for g in range(G):
                    col = ((mt * G + g) * 2 + h) * NSLOT
                    px = (mt * G + g) * TP
                    nc.tensor.matmul(
                        sp[32 * g:32 * g + 32, :],
                        KT_sb[:, col:col + NSLOT],
                        fT_t[h][:, px:px + TP],
                        start=(h == 0), stop=False,
                        tile_position=(0, 32 * g),
                        skip_group_check=True)
            # +BIG mask on own-instance slots: one K=16 block-diag matmul
            nc.tensor.matmul(
                sp[:], EB_sb[:, NMT * TP:], EB_sb[:, mt * TP:(mt + 1) * TP],
                start=False, stop=True, tile_position=(0, 0),
                skip_group_check=True)
            # one exp for all 2048 pixels of the group
            P = io.tile([128, TP], BF16, tag="p", bufs=2)
            nc.scalar.activation(P[:], sp[:], AF.Exp, bias=sbj_ap(mt))
            # attention: per pixel-chunk c, two pairs of row-group
            # concurrent matmuls into adjacent PSUM banks; paired
            # strided-AP copies (split vector/scalar) convert to bf16
            otb = io.tile([128, 16 * 257], BF16, tag="otb", bufs=2)
            for c in range(4):
                for gp in range(2):
                    op2 = ops.tile([128, 1024], F32, tag="o", bufs=3)
                    for k in range(2):
                        g = 2 * gp + k
                        nc.tensor.matmul(
                            op2[:, 512 * k:512 * k + 257],
                            P[32 * g:32 * g + 32, c * 128:(c + 1) * 128],
                            KV_sb[32 * g:32 * g + 32,
                                  V2_0 + mt * 257:V2_0 + (mt + 1) * 257],
                            start=True, stop=True,
                            tile_position=(32 * g, 0))
                    src = op2[:].rearrange("p (b x) -> p b x", b=2)[:, :, 0:257]
                    base = (c * 4 + 2 * gp) * 257
                    dst = otb[:, base:base + 514].rearrange(
                        "p (b x) -> p b x", b=2)
                    if (c * 2 + gp) % 2 == 0:
                        nc.vector.tensor_copy(dst, src)
                    else:
                        nc.scalar.copy(dst, src)
                # drain each ready half off-chip early (4KB lines)
                if c in (1, 3):
                    b0, b1 = (c - 1) * 4 * 257, (c + 1) * 4 * 257
                    nc.gpsimd.dma_start(OB[mt, :, b0:b1], otb[:, b0:b1])
        nc.scalar.dma_start(warm_dram[:, :], wsink[:])

    nc.compile()
    return nc


_CACHE = {}


def _build():
    if "nc" not in _CACHE:
        _CACHE["nc"] = build_nc()
    return _CACHE["nc"]


def _prep_maps(anchors, features, instances_in_view, in_proj_w, in_proj_b,
               out_w, out_b):
    f32 = np.float32
    bf16 = ml_dtypes.bfloat16
    anchors = np.asarray(anchors, f32)
    features = np.asarray(features, f32)
    iiv = np.asarray(instances_in_view, np.int32)
    in_proj_w = np.asarray(in_proj_w, f32)
    in_proj_b = np.asarray(in_proj_b, f32)
    out_w = np.asarray(out_w, f32)
    out_b = np.asarray(out_b, f32)

    f_flat = features.reshape(N_FULL, C)
    lab = iiv.reshape(-1)
    idx = np.maximum(lab - 1, 0)
    perm = np.argsort(idx, kind="stable")
    idx_s = idx[perm]
    f_s = f_flat[perm]
    fT_full = np.ascontiguousarray(f_s.T.astype(bf16))  # (C, N) sorted

    # replicated folded tables (see module docstring)
    A = anchors.reshape(J, C)
    Wq, Wk, Wv = in_proj_w[:C], in_proj_w[C:2 * C], in_proj_w[2 * C:]
    bq, bk, bv = in_proj_b[:C], in_proj_b[C:2 * C], in_proj_b[2 * C:]
    K_all = A @ Wk.T + bk                       # (J, C)
    KW = f32(SCALE) * (K_all @ Wq)              # (J, C)
    sb = f32(SCALE) * (K_all @ bq)              # (J,)
    V2_h = (A @ Wv.T + bv) @ out_w.T + out_b    # (J, C)

    NSB = N_FULL // TP  # 64 sub-blocks globally
    rows = np.zeros((NSB, NSLOT), np.int64)     # anchor row per slot
    valid = np.zeros((NSB, NSLOT), bool)
    uniq_pad = np.full((NSB, NI), -1, np.int64)
    for b in range(NSB):
        w = idx_s[b * TP:(b + 1) * TP]
        u = np.unique(w)
        assert len(u) <= NI, f"sub-block {b} spans {len(u)} instances"
        uniq_pad[b, :len(u)] = u
        for i, inst in enumerate(u):
            rows[b, i * L:(i + 1) * L] = inst * L + np.arange(L)
            valid[b, i * L:(i + 1) * L] = True

    KW_slot = KW[rows] * valid[:, :, None]          # (NSB, 32, C)
    sb_slot = sb[rows] * valid                      # (NSB, 32)
    V2_slot = np.zeros((NSB, NSLOT, 257), f32)
    V2_slot[:, :, :256] = V2_h[rows] * valid[:, :, None]
    V2_slot[:, :, 256] = valid

    # one-hot instance membership per pixel (4 rows per sub-block)
    eb = (idx_s.reshape(NSB, 1, TP) ==
          uniq_pad[:, :, None]).astype(bf16)        # (NSB, NI, TP)

    # block-diagonal mask lhsT: RC[4g+i, 32g'+s] = BIG iff g==g', s//8==i
    RC_h = np.zeros((G * NI, 128), f32)
    for g in range(G):
        for i in range(NI):
            RC_h[NI * g + i, 32 * g + i * L:32 * g + (i + 1) * L] = BIG
    RC_h = RC_h.astype(bf16)

    in_maps = []
    for ci in range(NCORES):
        sl = slice(ci * NP, (ci + 1) * NP)
        bsl = slice(ci * (NP // TP), (ci + 1) * (NP // TP))  # 8 sub-blocks
        kw_c = KW_slot[bsl].reshape(NMT, G, NSLOT, 2, 128)   # (mt,g,s,h,p)
        KT_h = np.ascontiguousarray(
            kw_c.transpose(4, 0, 1, 3, 2).reshape(128, NMT * G * 2 * NSLOT)
        ).astype(bf16)
        v2_c = V2_slot[bsl].reshape(NMT, G, NSLOT, 257)
        V2R_h = np.ascontiguousarray(
            v2_c.transpose(1, 2, 0, 3).reshape(128, NMT * 257)).astype(bf16)
        sb_c = sb_slot[bsl].reshape(NMT, G, NSLOT)
        SBJ_h = np.ascontiguousarray(
            sb_c.transpose(1, 2, 0).reshape(128, NMT).astype(f32))
        # bit-pack the f32 bias as trailing bf16 columns of V2R
        SBJ_bits = SBJ_h.view(np.uint16).view(bf16)          # [128, 2*NMT]
        eb_c = eb[bsl].reshape(NMT, G, NI, TP)
        EB_h = np.ascontiguousarray(
            eb_c.transpose(1, 2, 0, 3).reshape(G * NI, NMT * TP))
        in_maps.append({
            "fT": np.ascontiguousarray(fT_full[:, sl]),
            "KTV2": np.concatenate([KT_h, V2R_h, SBJ_bits], axis=1),
            "EBRC": np.concatenate([EB_h, RC_h], axis=1),
        })
    ctx = {"perm": perm, "f_s": f_s, "lab_s": lab[perm],
           "shape": features.shape}
    return in_maps, ctx


def _run(in_maps, **kw):
    nc = _build()
    return run_bass_kernel_spmd(nc, in_maps, core_ids=list(range(NCORES)),
                                **kw)


def kernel(**inputs):
    in_maps, ctx = _prep_maps(**inputs)
    res = _run(in_maps)
    o_parts = []
    for r in res.results:
        ob = np.asarray(r["OB"]).reshape(NMT, 128, 4, G, 257)
        o_parts.append(np.ascontiguousarray(
            ob.transpose(0, 3, 2, 1, 4)).reshape(NP, 257))
    o_cat = np.concatenate(o_parts, axis=0).astype(np.float32)
    o = o_cat[:, :256] / o_cat[:, 256:257]
    res_s = ctx["f_s"] + (ctx["lab_s"] > 0)[:, None] * o
    out = np.empty_like(res_s)
    out[ctx["perm"]] = res_s
    return out.reshape(ctx["shape"]).astype(np.float32)
